# revision 5
# baseline (speedup 1.0000x reference)
"""Trainium2 Bass kernel for nn_MetricLoss (segment_reduce / discriminative loss).

Reference math (K=32 labels, D=16):
  cents[s,k,:]  = mean of embeddings of sample s where label==k
  push[s]       = sum_{k<j} relu(0.25 - L1(c_sk, c_sj))^2 / 496
  pull[s]       = mean over ALL B*H*W pixels p of  L1(e_p, c_s,label_p)^2
  loss          = mean_s (push[s] + 0.1 * pull[s])

Strategy (8 cores, two launches):
  Launch A: per-core partial centroid sums+counts.
    - host precomputes one-hot in (group, tile, k) layout -> contiguous
      moving operand for the PE (strided rhs APs measured 3x slower)
    - PE: 84 groups; stationary = 7 pixel-tiles of [emb(16) ; ones(1)]
      (119 cols padded to 128), moving = the group's one-hot [128, 224];
      accumulated into one PSUM bank; diag blocks hold per-tile sums+counts.
    - chunked DMAs staged via pool-buffer rotation so the first chunks land
      early (all in-flight DMAs progress round-robin at packet granularity,
      so issuing everything up front delays the first chunk to ~the last).
    - host sums blocks across groups/cores -> cents [4,32,16]
  Launch B: pull + push.
    - PE computes diff = cents[b, label_p, :] - emb_p DIRECTLY:
      lhsT = BT[:, 128j:128j+128] with 96 contraction rows =
        [oh_A(32) ; embT_A(16) ; oh_B(32) ; embT_B(16)]  (two pixel halves)
      rhs  = rhsC [96, 128]: cols (u, d, b):
        cols 0-63  = [centsT_db ; -rep(I16) ; 0]   (half A)
        cols 64-127= [0 ; centsT_db ; -rep(I16)]   (half B)
      -> psum [128 pix, (u2, d16, b4)] = diff, one MM per 128 pixels.
    - ACT Abs evacuates psum with a d-outermost AP -> absd [P, 16, 64];
      every level of the DVE add-tree over d is then a dense contiguous
      bf16 op -> 2x mode.  A few pairs go via DVE tensor_reduce(abs)
      directly from psum to balance ACT vs DVE load.
    - tail: dist^2 (TT 2x) + reduce over pixels -> pacc2 [128, 2, 4]
    - push computed redundantly per core from tiny cent tables.
"""

import numpy as np
import ml_dtypes

import concourse.bass as bass
import concourse.bacc as bacc
import concourse.mybir as mybir
from concourse.tile import TileContext
from concourse.bass_utils import run_bass_kernel_spmd

BF16 = ml_dtypes.bfloat16
F32 = np.float32

# problem constants (hardcoded per contract)
B, H, W, D, K = 4, 384, 384, 16, 32
NCORES = 8
NPIX_TOT = B * H * W              # 589824
NPIX = NPIX_TOT // NCORES         # 73728 per core
P = 128                           # partitions
TC = NPIX // P                    # 576 pixel columns per partition
TCP = 588                         # padded to 7*84 for launch A grouping
NG = TCP // 7                     # 84 weight groups
GW = 7 * 17                       # 119 weight cols per group
WCOLS_PAD = 10016                 # GW*(NG-1) + 128 = 10005 -> pad
LAB_PAD = 100.0                   # pad label (!= any of 0..31)
NCHA = 7                          # launch A chunks (12 groups each)
GCH = NG // NCHA                  # 12 groups per chunk

# launch B geometry
NHALF = NPIX // 2                 # 36864 pixels per half
NJ = NHALF // P                   # 288 blocks of 128 pixels per half
NPAIR = NJ // 8                   # 36 psum double-banks (8 blocks each)
NCHB = 12                         # BT DMA chunks
JCH = NJ // NCHB                  # 24 blocks per chunk (3 pairs)
DVE_PAIRS = {2, 8, 14, 20, 26, 32}  # pairs evacuated via DVE reduce-direct

PUSH_MARGIN = 0.25
PUSH_W = 1.0
PULL_W = 0.1
NCMP = K * (K - 1) / 2.0

_built = {}


def _build_launch_a():
    nc = bacc.Bacc("TRN2", target_bir_lowering=False, debug=False)
    bf = mybir.dt.bfloat16
    f32 = mybir.dt.float32

    emb17 = nc.dram_tensor("emb17", [P, WCOLS_PAD], bf, kind="ExternalInput")
    ohA = nc.dram_tensor("ohA", [P, NG * 224], bf, kind="ExternalInput")
    outA = nc.dram_tensor("outA", [P, 224], f32, kind="ExternalOutput")

    with TileContext(nc) as tc:
        with (
            tc.tile_pool(name="sbuf", bufs=1) as pool,
            tc.tile_pool(name="work", bufs=3) as wpool,
            tc.tile_pool(name="psum", bufs=1, space="PSUM") as psum_pool,
        ):
            emb_sb = pool.tile([P, WCOLS_PAD], bf)

            # staged chunk loads: pool rotation (bufs=3) blocks the Sync queue
            # so only ~3 chunks are in flight and chunk 0 lands early
            ech = 1428  # 12 groups * 119
            ohc = []
            for c in range(NCHA):
                t = wpool.tile([P, GCH * 224], bf, tag="oha", bufs=3, name=f"ohc_{c}")
                nc.sync.dma_start(
                    out=t[:], in_=ohA.ap()[:, c * GCH * 224 : (c + 1) * GCH * 224]
                )
                ohc.append(t)
                e0 = c * ech
                e1 = WCOLS_PAD if c == NCHA - 1 else (c + 1) * ech
                nc.sync.dma_start(
                    out=emb_sb[:, e0:e1], in_=emb17.ap()[:, e0:e1]
                )

            ps = psum_pool.tile([P, 7, K], mybir.dt.float32)
            for g in range(NG):
                nc.tensor.matmul(
                    ps[:],
                    emb_sb[:, GW * g : GW * g + 128],
                    ohc[g // GCH][:, (g % GCH) * 224 : (g % GCH + 1) * 224],
                    start=(g == 0),
                    stop=(g == NG - 1),
                )

            evac = pool.tile([P, 7 * K], f32)
            nc.vector.tensor_copy(out=evac[:], in_=ps[:].rearrange("p a b -> p (a b)"))
            nc.sync.dma_start(out=outA.ap(), in_=evac[:])
    nc.compile()
    return nc


def _build_launch_b():
    nc = bacc.Bacc("TRN2", target_bir_lowering=False, debug=False)
    bf = mybir.dt.bfloat16
    f32 = mybir.dt.float32

    BTd = nc.dram_tensor("BT", [96, NJ * P], bf, kind="ExternalInput")
    rhsCd = nc.dram_tensor("rhsC", [96, 128], bf, kind="ExternalInput")
    cppd = nc.dram_tensor("cpp", [P, D], bf, kind="ExternalInput")
    cjdd = nc.dram_tensor("cjd", [P, K * D], bf, kind="ExternalInput")
    triud = nc.dram_tensor("triu", [P, K], bf, kind="ExternalInput")
    pacc2d = nc.dram_tensor("pacc2", [P, 8], f32, kind="ExternalOutput")
    pushpd = nc.dram_tensor("pushp", [P, 1], f32, kind="ExternalOutput")

    with TileContext(nc) as tc:
        with (
            tc.tile_pool(name="sbuf", bufs=1) as pool,
            tc.tile_pool(name="work", bufs=4) as wpool,
            tc.tile_pool(name="psum", bufs=4, space="PSUM") as psum_pool,
        ):
            rhsC = pool.tile([96, 128], bf)
            cpp_sb = pool.tile([P, D], bf)
            cjd_sb = pool.tile([P, K, D], bf)
            triu_sb = pool.tile([P, K], bf)
            dist = pool.tile([P, NJ, 2, 4], bf)
            sqb = pool.tile([P, NJ, 2, 4], bf)
            pacc2 = pool.tile([P, 2, 4], f32)
            pushp = pool.tile([P, 1], f32)

            nc.sync.dma_start(out=rhsC[:], in_=rhsCd.ap())
            nc.sync.dma_start(out=cpp_sb[:], in_=cppd.ap())
            nc.sync.dma_start(
                out=cjd_sb[:], in_=cjdd.ap().rearrange("p (a b) -> p a b", b=D)
            )
            nc.sync.dma_start(out=triu_sb[:], in_=triud.ap())

            btc = []
            for c in range(NCHB):
                t = wpool.tile([96, JCH * P], bf, tag="bt", bufs=3, name=f"bt_{c}")
                nc.sync.dma_start(
                    out=t[:], in_=BTd.ap()[:, c * JCH * P : (c + 1) * JCH * P]
                )
                btc.append(t)

            for i in range(NPAIR):
                ps = psum_pool.tile(
                    [P, 8, 128], mybir.dt.float32, tag="ps", name=f"ps_{i}"
                )
                for jj in range(8):
                    j = 8 * i + jj
                    nc.tensor.matmul(
                        ps[:, jj, :],
                        btc[j // JCH][:, (j % JCH) * P : (j % JCH + 1) * P],
                        rhsC[:],
                        start=True,
                        stop=True,
                    )
                dsl = dist[:, 8 * i : 8 * (i + 1), :, :]
                if i in DVE_PAIRS:
                    with nc.allow_low_precision("dist in bf16; error averages out"):
                        nc.vector.tensor_reduce(
                            out=dsl,
                            in_=ps[:].rearrange(
                                "p j (u d b) -> p j u b d", u=2, d=D, b=4
                            ),
                            axis=mybir.AxisListType.X,
                            op=mybir.AluOpType.add,
                            apply_absolute_value=True,
                        )
                else:
                    # d-outermost evac: every tree level is a dense 2D bf16 op
                    absd = wpool.tile([P, 16, 64], bf, tag="absd", name=f"absd_{i}")
                    nc.scalar.activation(
                        absd[:],
                        ps[:].rearrange("p j (u d b) -> p d j u b", u=2, d=D, b=4),
                        mybir.ActivationFunctionType.Abs,
                    )
                    t8 = wpool.tile([P, 8, 64], bf, tag="t8", name=f"t8_{i}")
                    nc.vector.tensor_tensor(
                        out=t8[:], in0=absd[:, 0:8, :], in1=absd[:, 8:16, :],
                        op=mybir.AluOpType.add,
                    )
                    t4 = wpool.tile([P, 4, 64], bf, tag="t4", name=f"t4_{i}")
                    nc.vector.tensor_tensor(
                        out=t4[:], in0=t8[:, 0:4, :], in1=t8[:, 4:8, :],
                        op=mybir.AluOpType.add,
                    )
                    t2 = wpool.tile([P, 2, 64], bf, tag="t2", name=f"t2_{i}")
                    nc.vector.tensor_tensor(
                        out=t2[:], in0=t4[:, 0:2, :], in1=t4[:, 2:4, :],
                        op=mybir.AluOpType.add,
                    )
                    nc.vector.tensor_tensor(
                        out=dsl, in0=t2[:, 0:1, :], in1=t2[:, 1:2, :],
                        op=mybir.AluOpType.add,
                    )

            # pull partial: pacc2[p, u, b] = sum_j dist^2
            nc.vector.tensor_tensor(
                out=sqb[:], in0=dist[:], in1=dist[:], op=mybir.AluOpType.mult
            )
            nc.vector.tensor_reduce(
                out=pacc2[:],
                in_=sqb[:].rearrange("p j u b -> p u b j"),
                axis=mybir.AxisListType.X,
                op=mybir.AluOpType.add,
            )
            nc.sync.dma_start(out=pacc2d.ap(), in_=pacc2[:].rearrange("p a b -> p (a b)"))

            # push (tiny, redundant per core): partitions p=(b,k)
            pd_diff = pool.tile([P, K, D], bf)
            nc.vector.tensor_tensor(
                out=pd_diff[:],
                in0=cpp_sb[:].unsqueeze(1).broadcast_to([P, K, D]),
                in1=cjd_sb[:],
                op=mybir.AluOpType.subtract,
            )
            pd = pool.tile([P, K], f32)
            nc.vector.tensor_reduce(
                out=pd[:],
                in_=pd_diff[:],
                axis=mybir.AxisListType.X,
                op=mybir.AluOpType.add,
                apply_absolute_value=True,
            )
            # relu(margin - d)^2 == min(d - margin, 0)^2
            m = pool.tile([P, K], f32)
            nc.vector.tensor_scalar(
                out=m[:],
                in0=pd[:],
                scalar1=PUSH_MARGIN,
                scalar2=0.0,
                op0=mybir.AluOpType.subtract,
                op1=mybir.AluOpType.min,
            )
            msq = pool.tile([P, K], f32)
            nc.vector.tensor_tensor(
                out=msq[:], in0=m[:], in1=m[:], op=mybir.AluOpType.mult
            )
            msqm = pool.tile([P, K], f32)
            nc.vector.tensor_tensor(
                out=msqm[:], in0=msq[:], in1=triu_sb[:], op=mybir.AluOpType.mult
            )
            nc.vector.tensor_reduce(
                out=pushp[:],
                in_=msqm[:],
                axis=mybir.AxisListType.X,
                op=mybir.AluOpType.add,
            )
            nc.sync.dma_start(out=pushpd.ap(), in_=pushp[:])
    nc.compile()
    return nc


def _get(name):
    if name not in _built:
        if name == "A":
            _built[name] = _build_launch_a()
        else:
            _built[name] = _build_launch_b()
    return _built[name]


def _prep_a(emb_flat, lab_flat):
    """emb_flat [NPIX_TOT, D] f32, lab_flat [NPIX_TOT] i32 -> per-core in_maps."""
    kk = np.arange(K, dtype=np.int32)
    in_maps = []
    for c in range(NCORES):
        e = emb_flat[c * NPIX : (c + 1) * NPIX].astype(BF16).reshape(P, TC, D)
        l = lab_flat[c * NPIX : (c + 1) * NPIX].reshape(P, TC)
        e17 = np.zeros((P, TCP, 17), dtype=BF16)
        e17[:, :TC, :D] = e
        e17[:, :, D] = BF16(1.0)
        w = np.zeros((P, WCOLS_PAD), dtype=BF16)
        w[:, : TCP * 17] = e17.reshape(P, TCP * 17)
        lb = np.full((P, TCP), int(LAB_PAD), dtype=np.int32)
        lb[:, :TC] = l
        oh = (lb[:, :, None] == kk[None, None, :]).astype(BF16)  # [P, 588, 32]
        in_maps.append({"emb17": w, "ohA": np.ascontiguousarray(oh.reshape(P, NG * 224))})
    return in_maps


def _reduce_a(results):
    """outA [8][P, 224] -> cents [B, K, D] float64, counts [B, K]."""
    sums = np.zeros((B, K, D), dtype=np.float64)
    cnts = np.zeros((B, K), dtype=np.float64)
    for c in range(NCORES):
        o = results[c]["outA"].astype(np.float64).reshape(P, 7, K)
        s = c // 2
        for j in range(7):
            blk = o[17 * j : 17 * j + 17, j, :]  # [17, K]
            sums[s] += blk[:D].T  # [K, D]
            cnts[s] += blk[D]
    cents = sums / np.maximum(cnts, 1.0)[:, :, None]
    cents = np.where(cnts[:, :, None] > 0, cents, 0.0)
    return cents, cnts


def _prep_b(emb_flat, lab_flat, cents):
    cb = cents.astype(F32)  # [B, K, D]
    # col order within a u-half: n = d*4 + b  (d outer, b inner)
    centsT_db = cb.transpose(1, 2, 0).reshape(K, D * B)  # [k, (d,b)]
    negI_db = -np.repeat(np.eye(D, dtype=F32), B, axis=1)  # [16, (d,b)]
    rhsC = np.zeros((96, 128), dtype=BF16)
    rhsC[0:32, 0:64] = centsT_db.astype(BF16)
    rhsC[32:48, 0:64] = negI_db.astype(BF16)
    rhsC[48:80, 64:128] = centsT_db.astype(BF16)
    rhsC[80:96, 64:128] = negI_db.astype(BF16)

    cpp = cb.reshape(P, D).astype(BF16)  # p = 32b + k
    cjd = np.zeros((P, K * D), dtype=BF16)
    for b in range(4):
        cjd[32 * b : 32 * b + 32, :] = np.broadcast_to(
            cb[b].reshape(1, K * D), (K, K * D)
        ).astype(BF16)
    triu = np.zeros((P, K), dtype=BF16)
    kk = np.arange(K)
    for b in range(4):
        triu[32 * b : 32 * b + 32, :] = (kk[None, :] > kk[:, None]).astype(BF16)

    karange = np.arange(K, dtype=np.int32)
    in_maps = []
    for c in range(NCORES):
        e = emb_flat[c * NPIX : (c + 1) * NPIX].astype(BF16).reshape(2, NHALF, D)
        l = lab_flat[c * NPIX : (c + 1) * NPIX].reshape(2, NHALF)
        BT = np.zeros((96, NJ * P), dtype=BF16)
        for u in range(2):
            base = 48 * u
            BT[base : base + 32] = (karange[:, None] == l[u][None, :]).astype(BF16)
            BT[base + 32 : base + 48] = np.ascontiguousarray(e[u].T)
        in_maps.append(
            {
                "BT": BT,
                "rhsC": rhsC.copy(),
                "cpp": cpp.copy(),
                "cjd": cjd.copy(),
                "triu": triu.copy(),
            }
        )
    return in_maps


def run_launches(embeddings, labels, trace=False, trace_kwargs=None):
    """Returns (loss_scalar, resA, resB) — resA/resB are BassKernelResults."""
    emb_flat = np.ascontiguousarray(np.asarray(embeddings), dtype=F32).reshape(
        NPIX_TOT, D
    )
    lab_flat = np.ascontiguousarray(np.asarray(labels), dtype=np.int32).reshape(
        NPIX_TOT
    )
    core_ids = list(range(NCORES))

    kwA = dict(trace=trace, **(trace_kwargs or {}))
    resA = run_bass_kernel_spmd(_get("A"), _prep_a(emb_flat, lab_flat), core_ids, **kwA)
    cents, _ = _reduce_a(resA.results)

    resB = run_bass_kernel_spmd(
        _get("B"), _prep_b(emb_flat, lab_flat, cents), core_ids, **kwA
    )
    pull = np.zeros(4, dtype=np.float64)
    for c in range(NCORES):
        pull += resB.results[c]["pacc2"].astype(np.float64).reshape(P, 2, 4).sum(
            axis=(0, 1)
        )
    pull /= NPIX_TOT

    pushp = resB.results[0]["pushp"].astype(np.float64).reshape(4, K).sum(axis=1)
    push = pushp / NCMP

    loss = np.mean(PUSH_W * push + PULL_W * pull)
    return np.array(loss, dtype=F32), resA, resB


def kernel(embeddings, labels):
    loss, _, _ = run_launches(embeddings, labels, trace=False)
    return loss


# revision 6
# speedup vs baseline: 1.1381x; 1.1381x over previous
"""Trainium2 Bass kernel for nn_MetricLoss (segment_reduce / discriminative loss).

Reference math (K=32 labels, D=16):
  cents[s,k,:]  = mean of embeddings of sample s where label==k
  push[s]       = sum_{k<j} relu(0.25 - L1(c_sk, c_sj))^2 / 496
  pull[s]       = mean over ALL B*H*W pixels p of  L1(e_p, c_s,label_p)^2
  loss          = mean_s (push[s] + 0.1 * pull[s])

Strategy (8 cores, two launches).  All model DMAs drain FIFO on one HW
queue that only starts ~8.7us into the launch, so inputs are chunked and
issued in consumption order (no artificial staging).

  Launch A: per-core partial centroid sums+counts.
    - host precomputes one-hot in (group, tile, k) layout -> contiguous
      moving operand for the PE (strided rhs APs measured 3x slower)
    - PE: 84 groups; stationary = 7 pixel-tiles of [emb(16) ; ones(1)]
      (119 cols padded to 128), moving = the group's one-hot [128, 224];
      accumulated into a single PSUM bank; diagonal blocks hold sums+counts.
    - host sums blocks across groups/cores -> cents [4,32,16]
  Launch B: pull + push, fp8 inputs (one-hot/+-1 exact; emb quantization
    bias on pull ~0.1%, well under the 2e-2 gate).
    - PE computes diff = cents[b, label_p, :] - emb_p DIRECTLY:
      lhsT = BT[:, 128j:128j+128] with 96 contraction rows =
        [oh_A(32) ; embT_A(16) ; oh_B(32) ; embT_B(16)]  (two pixel halves)
      rhs  = rhsC [96, 128], cols (u, d, b):
        cols 0-63  = [centsT_db ; -rep(I16) ; 0]   (half A)
        cols 64-127= [0 ; centsT_db ; -rep(I16)]   (half B)
      -> psum [128 pix, (u2, d16, b4)] = diff, one MM per 128 pixels.
    - psum handled in 4-bank tiles (16 blocks): ACT Abs evacuates with a
      d-outermost AP -> absd [P, 16, 128]; the DVE add-tree over d is then
      dense contiguous bf16 at every level -> 2x mode.  A few tiles go via
      DVE tensor_reduce(abs) straight from psum to balance ACT vs DVE.
    - tail: dist^2 (TT 2x) + reduce over pixels -> pacc2 [128, 2, 4]
    - push computed redundantly per core from tiny cent tables.
"""

import numpy as np
import ml_dtypes

import concourse.bass as bass
import concourse.bacc as bacc
import concourse.mybir as mybir
from concourse.tile import TileContext
from concourse.bass_utils import run_bass_kernel_spmd

BF16 = ml_dtypes.bfloat16
FP8 = ml_dtypes.float8_e4m3fn
F32 = np.float32

# problem constants (hardcoded per contract)
B, H, W, D, K = 4, 384, 384, 16, 32
NCORES = 8
NPIX_TOT = B * H * W              # 589824
NPIX = NPIX_TOT // NCORES         # 73728 per core
P = 128                           # partitions
TC = NPIX // P                    # 576 pixel columns per partition
TCP = 588                         # padded to 7*84 for launch A grouping
NG = TCP // 7                     # 84 weight groups
GW = 7 * 17                       # 119 weight cols per group
WCOLS_PAD = 10016                 # GW*(NG-1) + 128 = 10005 -> pad
LAB_PAD = 100.0                   # pad label (!= any of 0..31)
NCHA = 7                          # launch A chunks (12 groups each)
GCH = NG // NCHA                  # 12 groups per chunk

# launch B geometry
NHALF = NPIX // 2                 # 36864 pixels per half
NJ = NHALF // P                   # 288 blocks of 128 pixels per half
NQUAD = NJ // 16                  # 18 psum 4-bank tiles (16 blocks each)
NCHB = 12                         # BT DMA chunks
JCH = NJ // NCHB                  # 24 blocks per chunk
DVE_QUADS = {4, 10, 16}           # quads evacuated via DVE reduce-direct

PUSH_MARGIN = 0.25
PUSH_W = 1.0
PULL_W = 0.1
NCMP = K * (K - 1) / 2.0

_built = {}


def _build_launch_a():
    nc = bacc.Bacc("TRN2", target_bir_lowering=False, debug=False)
    bf = mybir.dt.bfloat16
    f32 = mybir.dt.float32

    emb17 = nc.dram_tensor("emb17", [P, WCOLS_PAD], bf, kind="ExternalInput")
    ohA = nc.dram_tensor("ohA", [P, NG * 224], mybir.dt.float8e4, kind="ExternalInput")
    outA = nc.dram_tensor("outA", [P, 224], f32, kind="ExternalOutput")

    with TileContext(nc) as tc:
        with (
            tc.tile_pool(name="sbuf", bufs=1) as pool,
            tc.tile_pool(name="psum", bufs=1, space="PSUM") as psum_pool,
        ):
            emb_sb = pool.tile([P, WCOLS_PAD], bf)
            oh_sb = pool.tile([P, NG * 224], mybir.dt.float8e4)

            ech = 1428  # 12 groups * 119
            for c in range(NCHA):
                o0, o1 = c * GCH * 224, (c + 1) * GCH * 224
                nc.sync.dma_start(out=oh_sb[:, o0:o1], in_=ohA.ap()[:, o0:o1])
                e0 = c * ech
                e1 = WCOLS_PAD if c == NCHA - 1 else (c + 1) * ech
                nc.sync.dma_start(out=emb_sb[:, e0:e1], in_=emb17.ap()[:, e0:e1])

            ps = psum_pool.tile([P, 7, K], mybir.dt.float32)
            for g in range(NG):
                nc.tensor.matmul(
                    ps[:],
                    emb_sb[:, GW * g : GW * g + 128],
                    oh_sb[:, 224 * g : 224 * (g + 1)],
                    start=(g == 0),
                    stop=(g == NG - 1),
                )

            evac = pool.tile([P, 7 * K], f32)
            nc.vector.tensor_copy(out=evac[:], in_=ps[:].rearrange("p a b -> p (a b)"))
            nc.sync.dma_start(out=outA.ap(), in_=evac[:])
    nc.compile()
    return nc


def _build_launch_b():
    nc = bacc.Bacc("TRN2", target_bir_lowering=False, debug=False)
    bf = mybir.dt.bfloat16
    f8 = mybir.dt.float8e4
    f32 = mybir.dt.float32

    BTd = nc.dram_tensor("BT", [96, NJ * P], f8, kind="ExternalInput")
    rhsCd = nc.dram_tensor("rhsC", [96, 128], f8, kind="ExternalInput")
    cppd = nc.dram_tensor("cpp", [P, D], bf, kind="ExternalInput")
    cjdd = nc.dram_tensor("cjd", [P, K * D], bf, kind="ExternalInput")
    triud = nc.dram_tensor("triu", [P, K], bf, kind="ExternalInput")
    pacc2d = nc.dram_tensor("pacc2", [P, 8], f32, kind="ExternalOutput")
    pushpd = nc.dram_tensor("pushp", [P, 1], f32, kind="ExternalOutput")

    with TileContext(nc) as tc:
        with (
            tc.tile_pool(name="sbuf", bufs=1) as pool,
            tc.tile_pool(name="work", bufs=4) as wpool,
            tc.tile_pool(name="psum", bufs=2, space="PSUM") as psum_pool,
        ):
            BT = pool.tile([96, NJ * P], f8)
            rhsC = pool.tile([96, 128], f8)
            cpp_sb = pool.tile([P, D], bf)
            cjd_sb = pool.tile([P, K, D], bf)
            triu_sb = pool.tile([P, K], bf)
            dist = pool.tile([P, NJ, 2, 4], bf)
            sqb = pool.tile([P, NJ, 2, 4], bf)
            pacc2 = pool.tile([P, 2, 4], f32)
            pushp = pool.tile([P, 1], f32)

            nc.sync.dma_start(out=rhsC[:], in_=rhsCd.ap())
            ch = JCH * P
            nc.sync.dma_start(out=BT[:, 0:ch], in_=BTd.ap()[:, 0:ch])
            nc.sync.dma_start(out=cpp_sb[:], in_=cppd.ap())
            nc.sync.dma_start(
                out=cjd_sb[:], in_=cjdd.ap().rearrange("p (a b) -> p a b", b=D)
            )
            nc.sync.dma_start(out=triu_sb[:], in_=triud.ap())
            for c in range(1, NCHB):
                nc.sync.dma_start(
                    out=BT[:, c * ch : (c + 1) * ch],
                    in_=BTd.ap()[:, c * ch : (c + 1) * ch],
                )

            for q in range(NQUAD):
                ps = psum_pool.tile(
                    [P, 16, 128], mybir.dt.float32, tag="ps", name=f"ps_{q}"
                )
                for jj in range(16):
                    j = 16 * q + jj
                    nc.tensor.matmul(
                        ps[:, jj, :],
                        BT[:, P * j : P * (j + 1)],
                        rhsC[:],
                        start=True,
                        stop=True,
                    )
                dsl = dist[:, 16 * q : 16 * (q + 1), :, :]
                if q in DVE_QUADS:
                    with nc.allow_low_precision("dist in bf16; error averages out"):
                        nc.vector.tensor_reduce(
                            out=dsl,
                            in_=ps[:].rearrange(
                                "p j (u d b) -> p j u b d", u=2, d=D, b=4
                            ),
                            axis=mybir.AxisListType.X,
                            op=mybir.AluOpType.add,
                            apply_absolute_value=True,
                        )
                else:
                    # d-outermost evac: every tree level is a dense 2D bf16 op
                    absd = wpool.tile([P, 16, 128], bf, tag="absd", name=f"absd_{q}")
                    nc.scalar.activation(
                        absd[:],
                        ps[:].rearrange("p j (u d b) -> p d j u b", u=2, d=D, b=4),
                        mybir.ActivationFunctionType.Abs,
                    )
                    t8 = wpool.tile([P, 8, 128], bf, tag="t8", name=f"t8_{q}")
                    nc.vector.tensor_tensor(
                        out=t8[:], in0=absd[:, 0:8, :], in1=absd[:, 8:16, :],
                        op=mybir.AluOpType.add,
                    )
                    t4 = wpool.tile([P, 4, 128], bf, tag="t4", name=f"t4_{q}")
                    nc.vector.tensor_tensor(
                        out=t4[:], in0=t8[:, 0:4, :], in1=t8[:, 4:8, :],
                        op=mybir.AluOpType.add,
                    )
                    t2 = wpool.tile([P, 2, 128], bf, tag="t2", name=f"t2_{q}")
                    nc.vector.tensor_tensor(
                        out=t2[:], in0=t4[:, 0:2, :], in1=t4[:, 2:4, :],
                        op=mybir.AluOpType.add,
                    )
                    nc.vector.tensor_tensor(
                        out=dsl, in0=t2[:, 0:1, :], in1=t2[:, 1:2, :],
                        op=mybir.AluOpType.add,
                    )

            # pull partial: pacc2[p, u, b] = sum_j dist^2
            nc.vector.tensor_tensor(
                out=sqb[:], in0=dist[:], in1=dist[:], op=mybir.AluOpType.mult
            )
            nc.vector.tensor_reduce(
                out=pacc2[:],
                in_=sqb[:].rearrange("p j u b -> p u b j"),
                axis=mybir.AxisListType.X,
                op=mybir.AluOpType.add,
            )
            nc.sync.dma_start(out=pacc2d.ap(), in_=pacc2[:].rearrange("p a b -> p (a b)"))

            # push (tiny, redundant per core): partitions p=(b,k)
            pd_diff = pool.tile([P, K, D], bf)
            nc.vector.tensor_tensor(
                out=pd_diff[:],
                in0=cpp_sb[:].unsqueeze(1).broadcast_to([P, K, D]),
                in1=cjd_sb[:],
                op=mybir.AluOpType.subtract,
            )
            pd = pool.tile([P, K], f32)
            nc.vector.tensor_reduce(
                out=pd[:],
                in_=pd_diff[:],
                axis=mybir.AxisListType.X,
                op=mybir.AluOpType.add,
                apply_absolute_value=True,
            )
            # relu(margin - d)^2 == min(d - margin, 0)^2
            m = pool.tile([P, K], f32)
            nc.vector.tensor_scalar(
                out=m[:],
                in0=pd[:],
                scalar1=PUSH_MARGIN,
                scalar2=0.0,
                op0=mybir.AluOpType.subtract,
                op1=mybir.AluOpType.min,
            )
            msq = pool.tile([P, K], f32)
            nc.vector.tensor_tensor(
                out=msq[:], in0=m[:], in1=m[:], op=mybir.AluOpType.mult
            )
            msqm = pool.tile([P, K], f32)
            nc.vector.tensor_tensor(
                out=msqm[:], in0=msq[:], in1=triu_sb[:], op=mybir.AluOpType.mult
            )
            nc.vector.tensor_reduce(
                out=pushp[:],
                in_=msqm[:],
                axis=mybir.AxisListType.X,
                op=mybir.AluOpType.add,
            )
            nc.sync.dma_start(out=pushpd.ap(), in_=pushp[:])
    nc.compile()
    return nc


def _get(name):
    if name not in _built:
        if name == "A":
            _built[name] = _build_launch_a()
        else:
            _built[name] = _build_launch_b()
    return _built[name]


def _prep_a(emb_flat, lab_flat):
    """emb_flat [NPIX_TOT, D] f32, lab_flat [NPIX_TOT] i32 -> per-core in_maps."""
    kk = np.arange(K, dtype=np.int32)
    in_maps = []
    for c in range(NCORES):
        e = emb_flat[c * NPIX : (c + 1) * NPIX].astype(BF16).reshape(P, TC, D)
        l = lab_flat[c * NPIX : (c + 1) * NPIX].reshape(P, TC)
        e17 = np.zeros((P, TCP, 17), dtype=BF16)
        e17[:, :TC, :D] = e
        e17[:, :, D] = BF16(1.0)
        w = np.zeros((P, WCOLS_PAD), dtype=BF16)
        w[:, : TCP * 17] = e17.reshape(P, TCP * 17)
        lb = np.full((P, TCP), int(LAB_PAD), dtype=np.int32)
        lb[:, :TC] = l
        oh = (lb[:, :, None] == kk[None, None, :]).astype(FP8)  # [P, 588, 32]
        in_maps.append({"emb17": w, "ohA": np.ascontiguousarray(oh.reshape(P, NG * 224))})
    return in_maps


def _reduce_a(results):
    """outA [8][P, 224] -> cents [B, K, D] float64, counts [B, K]."""
    sums = np.zeros((B, K, D), dtype=np.float64)
    cnts = np.zeros((B, K), dtype=np.float64)
    for c in range(NCORES):
        o = results[c]["outA"].astype(np.float64).reshape(P, 7, K)
        s = c // 2
        for j in range(7):
            blk = o[17 * j : 17 * j + 17, j, :]  # [17, K]
            sums[s] += blk[:D].T  # [K, D]
            cnts[s] += blk[D]
    cents = sums / np.maximum(cnts, 1.0)[:, :, None]
    cents = np.where(cnts[:, :, None] > 0, cents, 0.0)
    return cents, cnts


def _prep_b(emb_flat, lab_flat, cents):
    cb = cents.astype(F32)  # [B, K, D]
    # col order within a u-half: n = d*4 + b  (d outer, b inner)
    centsT_db = cb.transpose(1, 2, 0).reshape(K, D * B)  # [k, (d,b)]
    negI_db = -np.repeat(np.eye(D, dtype=F32), B, axis=1)  # [16, (d,b)]
    rhsC = np.zeros((96, 128), dtype=FP8)
    rhsC[0:32, 0:64] = centsT_db.astype(FP8)
    rhsC[32:48, 0:64] = negI_db.astype(FP8)
    rhsC[48:80, 64:128] = centsT_db.astype(FP8)
    rhsC[80:96, 64:128] = negI_db.astype(FP8)

    cpp = cb.reshape(P, D).astype(BF16)  # p = 32b + k
    cjd = np.zeros((P, K * D), dtype=BF16)
    for b in range(4):
        cjd[32 * b : 32 * b + 32, :] = np.broadcast_to(
            cb[b].reshape(1, K * D), (K, K * D)
        ).astype(BF16)
    triu = np.zeros((P, K), dtype=BF16)
    kk = np.arange(K)
    for b in range(4):
        triu[32 * b : 32 * b + 32, :] = (kk[None, :] > kk[:, None]).astype(BF16)

    karange = np.arange(K, dtype=np.int32)
    in_maps = []
    for c in range(NCORES):
        e = emb_flat[c * NPIX : (c + 1) * NPIX].astype(FP8).reshape(2, NHALF, D)
        l = lab_flat[c * NPIX : (c + 1) * NPIX].reshape(2, NHALF)
        BT = np.zeros((96, NJ * P), dtype=FP8)
        for u in range(2):
            base = 48 * u
            BT[base : base + 32] = (karange[:, None] == l[u][None, :]).astype(FP8)
            BT[base + 32 : base + 48] = np.ascontiguousarray(e[u].T)
        in_maps.append(
            {
                "BT": BT,
                "rhsC": rhsC.copy(),
                "cpp": cpp.copy(),
                "cjd": cjd.copy(),
                "triu": triu.copy(),
            }
        )
    return in_maps


def run_launches(embeddings, labels, trace=False, trace_kwargs=None):
    """Returns (loss_scalar, resA, resB) — resA/resB are BassKernelResults."""
    emb_flat = np.ascontiguousarray(np.asarray(embeddings), dtype=F32).reshape(
        NPIX_TOT, D
    )
    lab_flat = np.ascontiguousarray(np.asarray(labels), dtype=np.int32).reshape(
        NPIX_TOT
    )
    core_ids = list(range(NCORES))

    kwA = dict(trace=trace, **(trace_kwargs or {}))
    resA = run_bass_kernel_spmd(_get("A"), _prep_a(emb_flat, lab_flat), core_ids, **kwA)
    cents, _ = _reduce_a(resA.results)

    resB = run_bass_kernel_spmd(
        _get("B"), _prep_b(emb_flat, lab_flat, cents), core_ids, **kwA
    )
    pull = np.zeros(4, dtype=np.float64)
    for c in range(NCORES):
        pull += resB.results[c]["pacc2"].astype(np.float64).reshape(P, 2, 4).sum(
            axis=(0, 1)
        )
    pull /= NPIX_TOT

    pushp = resB.results[0]["pushp"].astype(np.float64).reshape(4, K).sum(axis=1)
    push = pushp / NCMP

    loss = np.mean(PUSH_W * push + PULL_W * pull)
    return np.array(loss, dtype=F32), resA, resB


def kernel(embeddings, labels):
    loss, _, _ = run_launches(embeddings, labels, trace=False)
    return loss


# revision 11
# speedup vs baseline: 1.1710x; 1.0289x over previous
"""Trainium2 Bass kernel for nn_MetricLoss (segment_reduce / discriminative loss).

Reference math (K=32 labels, D=16):
  cents[s,k,:]  = mean of embeddings of sample s where label==k
  push[s]       = sum_{k<j} relu(0.25 - L1(c_sk, c_sj))^2 / 496
  pull[s]       = mean over ALL B*H*W pixels p of  L1(e_p, c_s,label_p)^2
  loss          = mean_s (push[s] + 0.1 * pull[s])

Strategy (8 cores, two launches).  All model DMAs drain FIFO on one HW
queue that only starts ~8.7us into the launch, so inputs are chunked and
issued in consumption order (no artificial staging).

  Launch A: per-core partial centroid sums+counts.
    - host precomputes one-hot in (group, tile, k) layout -> contiguous
      moving operand for the PE (strided rhs APs measured 3x slower)
    - PE: 84 groups; stationary = 7 pixel-tiles of [emb(16) ; ones(1)]
      (119 cols padded to 128), moving = the group's one-hot [128, 224];
      accumulated into a single PSUM bank; diagonal blocks hold sums+counts.
    - host sums blocks across groups/cores -> cents [4,32,16]
  Launch B: pull + push, fp8 inputs (one-hot/+-1 exact; emb quantization
    bias on pull ~0.1%, well under the 2e-2 gate).
    - PE computes diff = cents[b, label_p, :] - emb_p DIRECTLY:
      lhsT = BT[:, 128j:128j+128] with 96 contraction rows =
        [oh_A(32) ; embT_A(16) ; oh_B(32) ; embT_B(16)]  (two pixel halves)
      rhs  = rhsC [96, 128], cols (u, d, b):
        cols 0-63  = [centsT_db ; -rep(I16) ; 0]   (half A)
        cols 64-127= [0 ; centsT_db ; -rep(I16)]   (half B)
      -> psum [128 pix, (u2, d16, b4)] = diff, one MM per 128 pixels.
    - psum handled in 4-bank tiles (16 blocks): ACT Abs evacuates with a
      d-outermost AP -> absd [P, 16, 128]; the DVE add-tree over d is then
      dense contiguous bf16 at every level -> 2x mode.  A few tiles go via
      DVE tensor_reduce(abs) straight from psum to balance ACT vs DVE.
    - tail: dist^2 (TT 2x) + reduce over pixels -> pacc2 [128, 2, 4]
    - push computed redundantly per core from tiny cent tables.
"""

import numpy as np
import ml_dtypes

import concourse.bass as bass
import concourse.bacc as bacc
import concourse.mybir as mybir
from concourse.tile import TileContext
from concourse.bass_utils import run_bass_kernel_spmd

BF16 = ml_dtypes.bfloat16
FP8 = ml_dtypes.float8_e4m3fn
F32 = np.float32

# problem constants (hardcoded per contract)
B, H, W, D, K = 4, 384, 384, 16, 32
NCORES = 8
NPIX_TOT = B * H * W              # 589824
NPIX = NPIX_TOT // NCORES         # 73728 per core
P = 128                           # partitions
TC = NPIX // P                    # 576 pixel columns per partition
TCP = 588                         # padded to 7*84 for launch A grouping
ECOLS = TC * 17                   # 9792 emb17 cols (t-major, 17 per tile)
NGA = TC // 4                     # 144 launch A groups of 4 tiles (4*32=128 weight cols)
NCHA = 8                          # launch A chunks (18 groups each)
GCHA = NGA // NCHA                # 18 groups per chunk

# launch B geometry
NHALF = NPIX // 2                 # 36864 pixels per half
NJ = NHALF // P                   # 288 blocks of 128 pixels per half
NQUAD = NJ // 16                  # 18 psum 4-bank tiles (16 blocks each)
NCHB = 12                         # BT DMA chunks
JCH = NJ // NCHB                  # 24 blocks per chunk
DVE_QUADS = {9}                   # quads evacuated via DVE reduce-direct

PUSH_MARGIN = 0.25
PUSH_W = 1.0
PULL_W = 0.1
NCMP = K * (K - 1) / 2.0

_built = {}


def _build_launch_a():
    nc = bacc.Bacc("TRN2", target_bir_lowering=False, debug=False)
    bf = mybir.dt.bfloat16
    f32 = mybir.dt.float32

    emb17 = nc.dram_tensor("emb17", [P, ECOLS], bf, kind="ExternalInput")
    ohA = nc.dram_tensor("ohA", [P, NGA * 128], mybir.dt.float8e4, kind="ExternalInput")
    outA = nc.dram_tensor("outA", [P, 68], f32, kind="ExternalOutput")

    with TileContext(nc) as tc:
        with (
            tc.tile_pool(name="sbuf", bufs=1) as pool,
            tc.tile_pool(name="psum", bufs=1, space="PSUM") as psum_pool,
        ):
            emb_sb = pool.tile([P, ECOLS], bf)
            oh_sb = pool.tile([P, NGA * 128], mybir.dt.float8e4)

            # one-hot (stationary) first in each chunk pair: it gates LDWEIGHTS
            for c in range(NCHA):
                o0, o1 = c * GCHA * 128, (c + 1) * GCHA * 128
                nc.sync.dma_start(out=oh_sb[:, o0:o1], in_=ohA.ap()[:, o0:o1])
                e0, e1 = c * GCHA * 68, (c + 1) * GCHA * 68
                nc.sync.dma_start(out=emb_sb[:, e0:e1], in_=emb17.ap()[:, e0:e1])

            # stationary = one-hot group [128, (4t,32k)], moving = emb17 [128, (4t,17d)]
            # diag blocks (t==t') accumulate over groups: block tau holds sums
            # over all pixels with tile-index t = 4g+tau
            ps = psum_pool.tile([P, 68], mybir.dt.float32)
            for g in range(NGA):
                nc.tensor.matmul(
                    ps[:],
                    oh_sb[:, 128 * g : 128 * (g + 1)],
                    emb_sb[:, 68 * g : 68 * (g + 1)],
                    start=(g == 0),
                    stop=(g == NGA - 1),
                )

            evac = pool.tile([P, 68], f32)
            nc.vector.tensor_copy(out=evac[:], in_=ps[:])
            nc.sync.dma_start(out=outA.ap(), in_=evac[:])
    nc.compile()
    return nc


def _build_launch_b():
    nc = bacc.Bacc("TRN2", target_bir_lowering=False, debug=False)
    bf = mybir.dt.bfloat16
    f8 = mybir.dt.float8e4
    f32 = mybir.dt.float32

    BTd = nc.dram_tensor("BT", [96, NJ * P], f8, kind="ExternalInput")
    rhsCd = nc.dram_tensor("rhsC", [96, 128], f8, kind="ExternalInput")
    cppd = nc.dram_tensor("cpp", [P, D], bf, kind="ExternalInput")
    cjdd = nc.dram_tensor("cjd", [P, K * D], bf, kind="ExternalInput")
    triud = nc.dram_tensor("triu", [P, K], bf, kind="ExternalInput")
    pacc2d = nc.dram_tensor("pacc2", [P, 8], f32, kind="ExternalOutput")
    pushpd = nc.dram_tensor("pushp", [P, 1], f32, kind="ExternalOutput")

    with TileContext(nc) as tc:
        with (
            tc.tile_pool(name="sbuf", bufs=1) as pool,
            tc.tile_pool(name="work", bufs=4) as wpool,
            tc.tile_pool(name="psum", bufs=2, space="PSUM") as psum_pool,
        ):
            BT = pool.tile([96, NJ * P], f8)
            rhsC = pool.tile([96, 128], f8)
            cpp_sb = pool.tile([P, D], bf)
            cjd_sb = pool.tile([P, K, D], bf)
            triu_sb = pool.tile([P, K], bf)
            dist = pool.tile([P, NJ, 2, 4], bf)
            sqb = pool.tile([P, NJ, 2, 4], bf)
            pacc2 = pool.tile([P, 2, 4], f32)
            pushp = pool.tile([P, 1], f32)

            nc.sync.dma_start(out=rhsC[:], in_=rhsCd.ap())
            ch = JCH * P
            nc.sync.dma_start(out=BT[:, 0:ch], in_=BTd.ap()[:, 0:ch])
            nc.sync.dma_start(out=cpp_sb[:], in_=cppd.ap())
            nc.sync.dma_start(
                out=cjd_sb[:], in_=cjdd.ap().rearrange("p (a b) -> p a b", b=D)
            )
            nc.sync.dma_start(out=triu_sb[:], in_=triud.ap())
            for c in range(1, NCHB):
                nc.sync.dma_start(
                    out=BT[:, c * ch : (c + 1) * ch],
                    in_=BTd.ap()[:, c * ch : (c + 1) * ch],
                )

            for q in range(NQUAD):
                ps = psum_pool.tile(
                    [P, 16, 128], mybir.dt.float32, tag="ps", name=f"ps_{q}"
                )
                for jj in range(16):
                    j = 16 * q + jj
                    nc.tensor.matmul(
                        ps[:, jj, :],
                        BT[:, P * j : P * (j + 1)],
                        rhsC[:],
                        start=True,
                        stop=True,
                    )
                dsl = dist[:, 16 * q : 16 * (q + 1), :, :]
                if q in DVE_QUADS:
                    with nc.allow_low_precision("dist in bf16; error averages out"):
                        nc.vector.tensor_reduce(
                            out=dsl,
                            in_=ps[:].rearrange(
                                "p j (u d b) -> p j u b d", u=2, d=D, b=4
                            ),
                            axis=mybir.AxisListType.X,
                            op=mybir.AluOpType.add,
                            apply_absolute_value=True,
                        )
                else:
                    # d-outermost evac: every tree level is a dense 2D bf16 op
                    absd = wpool.tile([P, 16, 128], bf, tag="absd", name=f"absd_{q}")
                    nc.scalar.activation(
                        absd[:],
                        ps[:].rearrange("p j (u d b) -> p d j u b", u=2, d=D, b=4),
                        mybir.ActivationFunctionType.Abs,
                    )
                    t8 = wpool.tile([P, 8, 128], bf, tag="t8", name=f"t8_{q}")
                    nc.vector.tensor_tensor(
                        out=t8[:], in0=absd[:, 0:8, :], in1=absd[:, 8:16, :],
                        op=mybir.AluOpType.add,
                    )
                    t4 = wpool.tile([P, 4, 128], bf, tag="t4", name=f"t4_{q}")
                    nc.vector.tensor_tensor(
                        out=t4[:], in0=t8[:, 0:4, :], in1=t8[:, 4:8, :],
                        op=mybir.AluOpType.add,
                    )
                    t2 = wpool.tile([P, 2, 128], bf, tag="t2", name=f"t2_{q}")
                    nc.vector.tensor_tensor(
                        out=t2[:], in0=t4[:, 0:2, :], in1=t4[:, 2:4, :],
                        op=mybir.AluOpType.add,
                    )
                    nc.vector.tensor_tensor(
                        out=dsl, in0=t2[:, 0:1, :], in1=t2[:, 1:2, :],
                        op=mybir.AluOpType.add,
                    )

            # pull partial: pacc2[p, u, b] = sum_j dist^2
            nc.vector.tensor_tensor(
                out=sqb[:], in0=dist[:], in1=dist[:], op=mybir.AluOpType.mult
            )
            nc.vector.tensor_reduce(
                out=pacc2[:],
                in_=sqb[:].rearrange("p j u b -> p u b j"),
                axis=mybir.AxisListType.X,
                op=mybir.AluOpType.add,
            )
            nc.sync.dma_start(out=pacc2d.ap(), in_=pacc2[:].rearrange("p a b -> p (a b)"))

            # push (tiny, redundant per core): partitions p=(b,k)
            pd_diff = pool.tile([P, K, D], bf)
            nc.vector.tensor_tensor(
                out=pd_diff[:],
                in0=cpp_sb[:].unsqueeze(1).broadcast_to([P, K, D]),
                in1=cjd_sb[:],
                op=mybir.AluOpType.subtract,
            )
            pd = pool.tile([P, K], f32)
            nc.vector.tensor_reduce(
                out=pd[:],
                in_=pd_diff[:],
                axis=mybir.AxisListType.X,
                op=mybir.AluOpType.add,
                apply_absolute_value=True,
            )
            # relu(margin - d)^2 == min(d - margin, 0)^2
            m = pool.tile([P, K], f32)
            nc.vector.tensor_scalar(
                out=m[:],
                in0=pd[:],
                scalar1=PUSH_MARGIN,
                scalar2=0.0,
                op0=mybir.AluOpType.subtract,
                op1=mybir.AluOpType.min,
            )
            msq = pool.tile([P, K], f32)
            nc.vector.tensor_tensor(
                out=msq[:], in0=m[:], in1=m[:], op=mybir.AluOpType.mult
            )
            msqm = pool.tile([P, K], f32)
            nc.vector.tensor_tensor(
                out=msqm[:], in0=msq[:], in1=triu_sb[:], op=mybir.AluOpType.mult
            )
            nc.vector.tensor_reduce(
                out=pushp[:],
                in_=msqm[:],
                axis=mybir.AxisListType.X,
                op=mybir.AluOpType.add,
            )
            nc.sync.dma_start(out=pushpd.ap(), in_=pushp[:])
    nc.compile()
    return nc


def _get(name):
    if name not in _built:
        if name == "A":
            _built[name] = _build_launch_a()
        else:
            _built[name] = _build_launch_b()
    return _built[name]


def _prep_a(emb_flat, lab_flat):
    """emb_flat [NPIX_TOT, D] f32, lab_flat [NPIX_TOT] i32 -> per-core in_maps."""
    kk = np.arange(K, dtype=np.int32)
    in_maps = []
    for c in range(NCORES):
        e = emb_flat[c * NPIX : (c + 1) * NPIX].astype(BF16).reshape(P, TC, D)
        l = lab_flat[c * NPIX : (c + 1) * NPIX].reshape(P, TC)
        e17 = np.zeros((P, TC, 17), dtype=BF16)
        e17[:, :, :D] = e
        e17[:, :, D] = BF16(1.0)
        oh = (l[:, :, None] == kk[None, None, :]).astype(FP8)  # [P, 576, 32]
        in_maps.append(
            {
                "emb17": np.ascontiguousarray(e17.reshape(P, ECOLS)),
                "ohA": np.ascontiguousarray(oh.reshape(P, NGA * 128)),
            }
        )
    return in_maps


def _reduce_a(results):
    """outA [8][P, 68] -> cents [B, K, D] float64, counts [B, K]."""
    sums = np.zeros((B, K, D), dtype=np.float64)
    cnts = np.zeros((B, K), dtype=np.float64)
    for c in range(NCORES):
        o = results[c]["outA"].astype(np.float64)  # [128=(4tau,32k), 68=(4tau,17)]
        s = c // 2
        for t in range(4):
            blk = o[32 * t : 32 * t + 32, 17 * t : 17 * t + 17]  # [K, 17]
            sums[s] += blk[:, :D]
            cnts[s] += blk[:, D]
    cents = sums / np.maximum(cnts, 1.0)[:, :, None]
    cents = np.where(cnts[:, :, None] > 0, cents, 0.0)
    return cents, cnts


def _prep_b(emb_flat, lab_flat, cents):
    cb = cents.astype(F32)  # [B, K, D]
    # col order within a u-half: n = d*4 + b  (d outer, b inner)
    centsT_db = cb.transpose(1, 2, 0).reshape(K, D * B)  # [k, (d,b)]
    negI_db = -np.repeat(np.eye(D, dtype=F32), B, axis=1)  # [16, (d,b)]
    rhsC = np.zeros((96, 128), dtype=FP8)
    rhsC[0:32, 0:64] = centsT_db.astype(FP8)
    rhsC[32:48, 0:64] = negI_db.astype(FP8)
    rhsC[48:80, 64:128] = centsT_db.astype(FP8)
    rhsC[80:96, 64:128] = negI_db.astype(FP8)

    cpp = cb.reshape(P, D).astype(BF16)  # p = 32b + k
    cjd = np.zeros((P, K * D), dtype=BF16)
    for b in range(4):
        cjd[32 * b : 32 * b + 32, :] = np.broadcast_to(
            cb[b].reshape(1, K * D), (K, K * D)
        ).astype(BF16)
    triu = np.zeros((P, K), dtype=BF16)
    kk = np.arange(K)
    for b in range(4):
        triu[32 * b : 32 * b + 32, :] = (kk[None, :] > kk[:, None]).astype(BF16)

    karange = np.arange(K, dtype=np.int32)
    in_maps = []
    for c in range(NCORES):
        e = emb_flat[c * NPIX : (c + 1) * NPIX].astype(FP8).reshape(2, NHALF, D)
        l = lab_flat[c * NPIX : (c + 1) * NPIX].reshape(2, NHALF)
        BT = np.zeros((96, NJ * P), dtype=FP8)
        for u in range(2):
            base = 48 * u
            BT[base : base + 32] = (karange[:, None] == l[u][None, :]).astype(FP8)
            BT[base + 32 : base + 48] = np.ascontiguousarray(e[u].T)
        in_maps.append(
            {
                "BT": BT,
                "rhsC": rhsC.copy(),
                "cpp": cpp.copy(),
                "cjd": cjd.copy(),
                "triu": triu.copy(),
            }
        )
    return in_maps


def run_launches(embeddings, labels, trace=False, trace_kwargs=None):
    """Returns (loss_scalar, resA, resB) — resA/resB are BassKernelResults."""
    emb_flat = np.ascontiguousarray(np.asarray(embeddings), dtype=F32).reshape(
        NPIX_TOT, D
    )
    lab_flat = np.ascontiguousarray(np.asarray(labels), dtype=np.int32).reshape(
        NPIX_TOT
    )
    core_ids = list(range(NCORES))

    kwA = dict(trace=trace, **(trace_kwargs or {}))
    resA = run_bass_kernel_spmd(_get("A"), _prep_a(emb_flat, lab_flat), core_ids, **kwA)
    cents, _ = _reduce_a(resA.results)

    resB = run_bass_kernel_spmd(
        _get("B"), _prep_b(emb_flat, lab_flat, cents), core_ids, **kwA
    )
    pull = np.zeros(4, dtype=np.float64)
    for c in range(NCORES):
        pull += resB.results[c]["pacc2"].astype(np.float64).reshape(P, 2, 4).sum(
            axis=(0, 1)
        )
    pull /= NPIX_TOT

    pushp = resB.results[0]["pushp"].astype(np.float64).reshape(4, K).sum(axis=1)
    push = pushp / NCMP

    loss = np.mean(PUSH_W * push + PULL_W * pull)
    return np.array(loss, dtype=F32), resA, resB


def kernel(embeddings, labels):
    loss, _, _ = run_launches(embeddings, labels, trace=False)
    return loss


# revision 19
# speedup vs baseline: 1.6320x; 1.3936x over previous
"""Trainium2 Bass kernel for nn_MetricLoss (segment_reduce / discriminative loss).

Reference math (K=32 labels, D=16):
  cents[s,k,:]  = mean of embeddings of sample s where label==k
  push[s]       = sum_{k<j} relu(0.25 - L1(c_sk, c_sj))^2 / 496
  pull[s]       = mean over ALL B*H*W pixels p of  L1(e_p, c_s,label_p)^2
  loss          = mean_s (push[s] + 0.1 * pull[s])

Strategy (8 cores, two launches).  All model DMAs drain FIFO on one HW
queue that only starts ~8.7us into the launch, so inputs are chunked and
issued in consumption order (no artificial staging).

  Launch A: per-core partial centroid sums+counts.
    - host precomputes one-hot in (group, tile, k) layout -> contiguous
      moving operand for the PE (strided rhs APs measured 3x slower)
    - PE: 84 groups; stationary = 7 pixel-tiles of [emb(16) ; ones(1)]
      (119 cols padded to 128), moving = the group's one-hot [128, 224];
      accumulated into a single PSUM bank; diagonal blocks hold sums+counts.
    - host sums blocks across groups/cores -> cents [4,32,16]
  Launch B: pull + push, fp8 inputs (one-hot/+-1 exact; emb quantization
    bias on pull ~0.1%, well under the 2e-2 gate).
    - PE computes diff = cents[b, label_p, :] - emb_p DIRECTLY:
      lhsT = BT[:, 128j:128j+128] with 96 contraction rows =
        [oh_A(32) ; embT_A(16) ; oh_B(32) ; embT_B(16)]  (two pixel halves)
      rhs  = rhsC [96, 128], cols (u, d, b):
        cols 0-63  = [centsT_db ; -rep(I16) ; 0]   (half A)
        cols 64-127= [0 ; centsT_db ; -rep(I16)]   (half B)
      -> psum [128 pix, (u2, d16, b4)] = diff, one MM per 128 pixels.
    - psum handled in 4-bank tiles (16 blocks): ACT Abs evacuates with a
      d-outermost AP -> absd [P, 16, 128]; the DVE add-tree over d is then
      dense contiguous bf16 at every level -> 2x mode.  A few tiles go via
      DVE tensor_reduce(abs) straight from psum to balance ACT vs DVE.
    - tail: dist^2 (TT 2x) + reduce over pixels -> pacc2 [128, 2, 4]
    - push computed redundantly per core from tiny cent tables.
"""

import numpy as np
import ml_dtypes

import concourse.bass as bass
import concourse.bacc as bacc
import concourse.mybir as mybir
from concourse.tile import TileContext
from concourse.bass_utils import run_bass_kernel_spmd

BF16 = ml_dtypes.bfloat16
FP8 = ml_dtypes.float8_e4m3fn
F32 = np.float32

# problem constants (hardcoded per contract)
B, H, W, D, K = 4, 384, 384, 16, 32
NCORES = 8
NPIX_TOT = B * H * W              # 589824
NPIX = NPIX_TOT // NCORES         # 73728 per core
P = 128                           # partitions
TC = NPIX // P                    # 576 pixel columns per partition
TCP = 588                         # padded to 7*84 for launch A grouping
ECOLS = TC * 17                   # 9792 emb17 cols (t-major, 17 per tile)
NGA = TC // 4                     # 144 launch A groups of 4 tiles (4*32=128 weight cols)
NCHA = 8                          # launch A chunks (18 groups each)
GCHA = NGA // NCHA                # 18 groups per chunk

# launch B geometry
NHALF = NPIX // 2                 # 36864 pixels per half
NJ = NHALF // P                   # 288 blocks of 128 pixels per half
NQUAD = NJ // 16                  # 18 psum 4-bank tiles (16 blocks each)
NCHB = 12                         # BT DMA chunks
JCH = NJ // NCHB                  # 24 blocks per chunk
DVE_QUADS = {8, 17}               # quads evacuated via DVE reduce-direct

PUSH_MARGIN = 0.25
PUSH_W = 1.0
PULL_W = 0.1
NCMP = K * (K - 1) / 2.0

_built = {}


def _build_launch_a():
    nc = bacc.Bacc("TRN2", target_bir_lowering=False, debug=False)
    bf = mybir.dt.bfloat16
    f32 = mybir.dt.float32

    emb17 = nc.dram_tensor("emb17", [P, ECOLS], mybir.dt.float8e4, kind="ExternalInput")
    ohA = nc.dram_tensor("ohA", [P, NGA * 128], mybir.dt.float8e4, kind="ExternalInput")
    outA = nc.dram_tensor("outA", [P, 68], f32, kind="ExternalOutput")

    with TileContext(nc) as tc:
        with (
            tc.tile_pool(name="sbuf", bufs=1) as pool,
            tc.tile_pool(name="psum", bufs=1, space="PSUM") as psum_pool,
        ):
            emb_sb = pool.tile([P, ECOLS], mybir.dt.float8e4)
            oh_sb = pool.tile([P, NGA * 128], mybir.dt.float8e4)

            # one-hot (stationary) first in each chunk pair: it gates LDWEIGHTS
            for c in range(NCHA):
                o0, o1 = c * GCHA * 128, (c + 1) * GCHA * 128
                nc.sync.dma_start(out=oh_sb[:, o0:o1], in_=ohA.ap()[:, o0:o1])
                e0, e1 = c * GCHA * 68, (c + 1) * GCHA * 68
                nc.sync.dma_start(out=emb_sb[:, e0:e1], in_=emb17.ap()[:, e0:e1])

            # stationary = one-hot group [128, (4t,32k)], moving = emb17 [128, (4t,17d)]
            # diag blocks (t==t') accumulate over groups: block tau holds sums
            # over all pixels with tile-index t = 4g+tau
            ps = psum_pool.tile([P, 68], mybir.dt.float32)
            for g in range(NGA):
                nc.tensor.matmul(
                    ps[:],
                    oh_sb[:, 128 * g : 128 * (g + 1)],
                    emb_sb[:, 68 * g : 68 * (g + 1)],
                    start=(g == 0),
                    stop=(g == NGA - 1),
                )

            evac = pool.tile([P, 68], f32)
            nc.vector.tensor_copy(out=evac[:], in_=ps[:])
            nc.sync.dma_start(out=outA.ap(), in_=evac[:])
    nc.compile()
    return nc


def _build_launch_b():
    nc = bacc.Bacc("TRN2", target_bir_lowering=False, debug=False)
    bf = mybir.dt.bfloat16
    f8 = mybir.dt.float8e4
    f32 = mybir.dt.float32

    BTd = nc.dram_tensor("BT", [96, NJ * P], f8, kind="ExternalInput")
    rhsCd = nc.dram_tensor("rhsC", [96, 128], f8, kind="ExternalInput")
    cppd = nc.dram_tensor("cpp", [P, D], bf, kind="ExternalInput")
    cjdd = nc.dram_tensor("cjd", [P, K * D], bf, kind="ExternalInput")
    triud = nc.dram_tensor("triu", [P, K], bf, kind="ExternalInput")
    pacc2d = nc.dram_tensor("pacc2", [P, 24], f32, kind="ExternalOutput")
    pushpd = nc.dram_tensor("pushp", [P, 1], f32, kind="ExternalOutput")

    with TileContext(nc) as tc:
        with (
            tc.tile_pool(name="sbuf", bufs=1) as pool,
            tc.tile_pool(name="work", bufs=4) as wpool,
            tc.tile_pool(name="psum", bufs=2, space="PSUM") as psum_pool,
        ):
            BT = pool.tile([96, NJ * P], f8)
            rhsC = pool.tile([96, 128], f8)
            cpp_sb = pool.tile([P, D], bf)
            cjd_sb = pool.tile([P, K, D], bf)
            triu_sb = pool.tile([P, K], bf)
            dist = pool.tile([P, NJ, 2, 4], bf)
            sqb = pool.tile([P, NJ, 2, 4], bf)
            pacc2 = pool.tile([P, 3, 2, 4], f32)
            pushp = pool.tile([P, 1], f32)

            nc.sync.dma_start(out=rhsC[:], in_=rhsCd.ap())
            ch = JCH * P
            nc.sync.dma_start(out=BT[:, 0:ch], in_=BTd.ap()[:, 0:ch])
            nc.sync.dma_start(out=cpp_sb[:], in_=cppd.ap())
            nc.sync.dma_start(
                out=cjd_sb[:], in_=cjdd.ap().rearrange("p (a b) -> p a b", b=D)
            )
            nc.sync.dma_start(out=triu_sb[:], in_=triud.ap())
            for c in range(1, NCHB):
                nc.sync.dma_start(
                    out=BT[:, c * ch : (c + 1) * ch],
                    in_=BTd.ap()[:, c * ch : (c + 1) * ch],
                )

            for q in range(NQUAD):
                ps = psum_pool.tile(
                    [P, 16, 128], mybir.dt.float32, tag="ps", name=f"ps_{q}"
                )
                for jj in range(16):
                    j = 16 * q + jj
                    nc.tensor.matmul(
                        ps[:, jj, :],
                        BT[:, P * j : P * (j + 1)],
                        rhsC[:],
                        start=True,
                        stop=True,
                    )
                dsl = dist[:, 16 * q : 16 * (q + 1), :, :]
                if q in DVE_QUADS:
                    with nc.allow_low_precision("dist in bf16; error averages out"):
                        nc.vector.tensor_reduce(
                            out=dsl,
                            in_=ps[:].rearrange(
                                "p j (u d b) -> p j u b d", u=2, d=D, b=4
                            ),
                            axis=mybir.AxisListType.X,
                            op=mybir.AluOpType.add,
                            apply_absolute_value=True,
                        )
                else:
                    # d-outermost evac: every tree level is a dense 2D bf16 op
                    absd = wpool.tile([P, 16, 128], bf, tag="absd", name=f"absd_{q}")
                    nc.scalar.activation(
                        absd[:],
                        ps[:].rearrange("p j (u d b) -> p d j u b", u=2, d=D, b=4),
                        mybir.ActivationFunctionType.Abs,
                    )
                    t8 = wpool.tile([P, 8, 128], bf, tag="t8", name=f"t8_{q}")
                    nc.vector.tensor_tensor(
                        out=t8[:], in0=absd[:, 0:8, :], in1=absd[:, 8:16, :],
                        op=mybir.AluOpType.add,
                    )
                    t4 = wpool.tile([P, 4, 128], bf, tag="t4", name=f"t4_{q}")
                    nc.vector.tensor_tensor(
                        out=t4[:], in0=t8[:, 0:4, :], in1=t8[:, 4:8, :],
                        op=mybir.AluOpType.add,
                    )
                    t2 = wpool.tile([P, 2, 128], bf, tag="t2", name=f"t2_{q}")
                    nc.vector.tensor_tensor(
                        out=t2[:], in0=t4[:, 0:2, :], in1=t4[:, 2:4, :],
                        op=mybir.AluOpType.add,
                    )
                    nc.vector.tensor_tensor(
                        out=dsl, in0=t2[:, 0:1, :], in1=t2[:, 1:2, :],
                        op=mybir.AluOpType.add,
                    )

                # pull partial in thirds (keeps the reduce off the tail):
                # pacc2[p, t, u, b] = sum_{j in third t} dist^2
                if q in (5, 11, 17):
                    t = (q + 1) // 6 - 1
                    j0, j1 = 96 * t, 96 * (t + 1)
                    nc.vector.tensor_tensor(
                        out=sqb[:, j0:j1, :, :],
                        in0=dist[:, j0:j1, :, :],
                        in1=dist[:, j0:j1, :, :],
                        op=mybir.AluOpType.mult,
                    )
                    nc.vector.tensor_reduce(
                        out=pacc2[:, t, :, :],
                        in_=sqb[:, j0:j1, :, :].rearrange("p j u b -> p u b j"),
                        axis=mybir.AxisListType.X,
                        op=mybir.AluOpType.add,
                    )

            nc.sync.dma_start(
                out=pacc2d.ap(), in_=pacc2[:].rearrange("p a b c -> p (a b c)")
            )

            # push (tiny, redundant per core): partitions p=(b,k)
            pd_diff = pool.tile([P, K, D], bf)
            nc.vector.tensor_tensor(
                out=pd_diff[:],
                in0=cpp_sb[:].unsqueeze(1).broadcast_to([P, K, D]),
                in1=cjd_sb[:],
                op=mybir.AluOpType.subtract,
            )
            pd = pool.tile([P, K], f32)
            nc.vector.tensor_reduce(
                out=pd[:],
                in_=pd_diff[:],
                axis=mybir.AxisListType.X,
                op=mybir.AluOpType.add,
                apply_absolute_value=True,
            )
            # relu(margin - d)^2 == min(d - margin, 0)^2
            m = pool.tile([P, K], f32)
            nc.vector.tensor_scalar(
                out=m[:],
                in0=pd[:],
                scalar1=PUSH_MARGIN,
                scalar2=0.0,
                op0=mybir.AluOpType.subtract,
                op1=mybir.AluOpType.min,
            )
            msq = pool.tile([P, K], f32)
            nc.vector.tensor_tensor(
                out=msq[:], in0=m[:], in1=m[:], op=mybir.AluOpType.mult
            )
            msqm = pool.tile([P, K], f32)
            nc.vector.tensor_tensor(
                out=msqm[:], in0=msq[:], in1=triu_sb[:], op=mybir.AluOpType.mult
            )
            nc.vector.tensor_reduce(
                out=pushp[:],
                in_=msqm[:],
                axis=mybir.AxisListType.X,
                op=mybir.AluOpType.add,
            )
            nc.sync.dma_start(out=pushpd.ap(), in_=pushp[:])
    nc.compile()
    return nc


def _get(name):
    if name not in _built:
        if name == "A":
            _built[name] = _build_launch_a()
        else:
            _built[name] = _build_launch_b()
    return _built[name]


def _prep_a(emb_flat, lab_flat):
    """emb_flat [NPIX_TOT, D] f32, lab_flat [NPIX_TOT] i32 -> per-core in_maps."""
    kk = np.arange(K, dtype=np.int32)
    in_maps = []
    for c in range(NCORES):
        e = emb_flat[c * NPIX : (c + 1) * NPIX].astype(BF16).reshape(P, TC, D)
        l = lab_flat[c * NPIX : (c + 1) * NPIX].reshape(P, TC)
        e17 = np.zeros((P, TC, 17), dtype=FP8)
        e17[:, :, :D] = e.astype(FP8)
        e17[:, :, D] = FP8(1.0)
        oh = (l[:, :, None] == kk[None, None, :]).astype(FP8)  # [P, 576, 32]
        in_maps.append(
            {
                "emb17": np.ascontiguousarray(e17.reshape(P, ECOLS)),
                "ohA": np.ascontiguousarray(oh.reshape(P, NGA * 128)),
            }
        )
    return in_maps


def _reduce_a(results):
    """outA [8][P, 68] -> cents [B, K, D] float64, counts [B, K]."""
    sums = np.zeros((B, K, D), dtype=np.float64)
    cnts = np.zeros((B, K), dtype=np.float64)
    for c in range(NCORES):
        o = results[c]["outA"].astype(np.float64)  # [128=(4tau,32k), 68=(4tau,17)]
        s = c // 2
        for t in range(4):
            blk = o[32 * t : 32 * t + 32, 17 * t : 17 * t + 17]  # [K, 17]
            sums[s] += blk[:, :D]
            cnts[s] += blk[:, D]
    cents = sums / np.maximum(cnts, 1.0)[:, :, None]
    cents = np.where(cnts[:, :, None] > 0, cents, 0.0)
    return cents, cnts


def _prep_b(emb_flat, lab_flat, cents):
    cb = cents.astype(F32)  # [B, K, D]
    # col order within a u-half: n = d*4 + b  (d outer, b inner)
    centsT_db = cb.transpose(1, 2, 0).reshape(K, D * B)  # [k, (d,b)]
    negI_db = -np.repeat(np.eye(D, dtype=F32), B, axis=1)  # [16, (d,b)]
    rhsC = np.zeros((96, 128), dtype=FP8)
    rhsC[0:32, 0:64] = centsT_db.astype(FP8)
    rhsC[32:48, 0:64] = negI_db.astype(FP8)
    rhsC[48:80, 64:128] = centsT_db.astype(FP8)
    rhsC[80:96, 64:128] = negI_db.astype(FP8)

    cpp = cb.reshape(P, D).astype(BF16)  # p = 32b + k
    cjd = np.zeros((P, K * D), dtype=BF16)
    for b in range(4):
        cjd[32 * b : 32 * b + 32, :] = np.broadcast_to(
            cb[b].reshape(1, K * D), (K, K * D)
        ).astype(BF16)
    triu = np.zeros((P, K), dtype=BF16)
    kk = np.arange(K)
    for b in range(4):
        triu[32 * b : 32 * b + 32, :] = (kk[None, :] > kk[:, None]).astype(BF16)

    karange = np.arange(K, dtype=np.int32)
    in_maps = []
    for c in range(NCORES):
        e = emb_flat[c * NPIX : (c + 1) * NPIX].astype(FP8).reshape(2, NHALF, D)
        l = lab_flat[c * NPIX : (c + 1) * NPIX].reshape(2, NHALF)
        BT = np.zeros((96, NJ * P), dtype=FP8)
        for u in range(2):
            base = 48 * u
            BT[base : base + 32] = (karange[:, None] == l[u][None, :]).astype(FP8)
            BT[base + 32 : base + 48] = np.ascontiguousarray(e[u].T)
        in_maps.append(
            {
                "BT": BT,
                "rhsC": rhsC.copy(),
                "cpp": cpp.copy(),
                "cjd": cjd.copy(),
                "triu": triu.copy(),
            }
        )
    return in_maps


def run_launches(embeddings, labels, trace=False, trace_kwargs=None):
    """Returns (loss_scalar, resA, resB) — resA/resB are BassKernelResults."""
    emb_flat = np.ascontiguousarray(np.asarray(embeddings), dtype=F32).reshape(
        NPIX_TOT, D
    )
    lab_flat = np.ascontiguousarray(np.asarray(labels), dtype=np.int32).reshape(
        NPIX_TOT
    )
    core_ids = list(range(NCORES))

    kwA = dict(trace=trace, **(trace_kwargs or {}))
    resA = run_bass_kernel_spmd(_get("A"), _prep_a(emb_flat, lab_flat), core_ids, **kwA)
    cents, _ = _reduce_a(resA.results)

    resB = run_bass_kernel_spmd(
        _get("B"), _prep_b(emb_flat, lab_flat, cents), core_ids, **kwA
    )
    pull = np.zeros(4, dtype=np.float64)
    for c in range(NCORES):
        pull += resB.results[c]["pacc2"].astype(np.float64).reshape(P, 3, 2, 4).sum(
            axis=(0, 1, 2)
        )
    pull /= NPIX_TOT

    pushp = resB.results[0]["pushp"].astype(np.float64).reshape(4, K).sum(axis=1)
    push = pushp / NCMP

    loss = np.mean(PUSH_W * push + PULL_W * pull)
    return np.array(loss, dtype=F32), resA, resB


def kernel(embeddings, labels):
    loss, _, _ = run_launches(embeddings, labels, trace=False)
    return loss
